# revision 1
# baseline (speedup 1.0000x reference)
"""AxisAttention kernel for 8 TRN2 NeuronCores.

Math (per batch b, per slice c in [0,64) with c = head*16 + hd_channel):
  qkv = W_qkv @ x_b (per pixel 1x1 conv) + b_qkv
  att_c[i,j] = sum_h q_c[h,i] k_c[h,j]              (contract over h)
  att = softmax((att + pos_c) * gamma, axis=j)
  net_c[h,j] = sum_i v_c[h,i] att[i,j]              (contract over i=w)

Design:
  - Data parallel: 1 batch per core, 8 cores, no collectives.
  - Two passes through DRAM spill buffers: pass 1 projects (channels on
    partitions, pixels on free dim) and writes qkv slice-major; pass 2
    reads per-slice (h, w) maps (the DRAM round trip performs the
    (ch,pix)->(h,w) partition transposition for free).
  - Spill is bf16 (halves spill traffic; rel err ~3e-3). SPILL_BF16=False
    reverts to float32r end-to-end (rel err ~3e-4, more DMA).
  - v is loaded back TRANSPOSED via the XBAR dma transpose (bf16-only
    feature), so the net matmul needs no on-chip transposes.
  - pos is never materialized: pos_c[i,j] = sin/cos(a_i + b_j) is rank-2
    by angle addition, folded into the attention matmul as 2 extra
    contraction rows with host-precomputed factors (float32r matmul).
  - Softmax needs no max subtraction (logits are bounded ~ +-2) and the
    ACT Exp writes psum->sbuf with scale=gamma, its accum_out giving the
    row sum for free.
  - DMA traffic is split across both HWDGE rings (nc.sync / nc.scalar)
    by parity, with output writes on the SWDGE (gpsimd) path.
"""

import sys

sys.path.insert(0, "/opt/trn_rl_repo")

import numpy as np

DIM = 64
NUM_HEAD = 4
SEQ = 256
HW = SEQ * SEQ  # 65536
BASE = 10000.0
GAMMA = (DIM // NUM_HEAD) ** (-0.5)  # 0.25
N_CORES = 8

SPILL_BF16 = True  # bf16 spill + bf16 attention matmuls (else float32r)
VT_XBAR = False  # vT via XBAR dma transpose (else PE transpose)
POS_BF16 = False  # pos factor matmul in bf16 (else float32r)
SG = 4  # pass-2 slice group size (slices per DMA batch)
X_BF16 = True  # x and W in bf16 (halves x read traffic)
import os as _os
PASSES = int(_os.environ.get("KERNEL_PASSES", "3"))  # bit0: pass1, bit1: pass2
OUT_BF16 = True  # device-side out in bf16 (host converts back to f32)

_CACHE = {}


def _pos_factors():
    """Rank-2 factorization of the additive positional map.

    pos[c, i, j] = emb[i*SEQ + j, c] with, for t = c // 2:
      c even: sin((i*SEQ + j) * inv_t)
      c odd:  cos((i*SEQ + j) * inv_t)
    With a = i*SEQ*inv_t, b = j*inv_t:
      sin(a+b) = sin a * cos b + cos a * sin b
      cos(a+b) = cos a * cos b + (-sin a) * sin b
    Returns posu, posw (64, 2, 256) float32 with pos[c] = posu[c].T @ posw[c].
    """
    half = DIM // 2
    inv = np.power(BASE, -2.0 * np.arange(half, dtype=np.float64) / DIM)
    i_idx = np.arange(SEQ, dtype=np.float64)
    posu = np.zeros((DIM, 2, SEQ), dtype=np.float64)
    posw = np.zeros((DIM, 2, SEQ), dtype=np.float64)
    two_pi = 2.0 * np.pi
    for c in range(DIM):
        t = c // 2
        a = np.mod(i_idx * SEQ * inv[t], two_pi)
        b = np.mod(i_idx * inv[t], two_pi)
        sa, ca = np.sin(a), np.cos(a)
        sb, cb = np.sin(b), np.cos(b)
        if c % 2 == 0:
            posu[c, 0], posu[c, 1] = sa, ca
        else:
            posu[c, 0], posu[c, 1] = ca, -sa
        posw[c, 0], posw[c, 1] = cb, sb
    return posu.astype(np.float32), posw.astype(np.float32)


def _build_nc(reps=1):
    import concourse.tile as tile
    from concourse import bacc, mybir
    from concourse.mybir import ActivationFunctionType as AF

    f32 = mybir.dt.float32
    f32r = mybir.dt.float32r
    bf16 = mybir.dt.bfloat16
    sdt = bf16 if SPILL_BF16 else f32r  # spill / attention dtype

    nc = bacc.Bacc(None, target_bir_lowering=False, debug=False)

    xdt = bf16 if X_BF16 else f32r
    odt = bf16 if OUT_BF16 else f32
    x_d = nc.declare_dram_parameter("x", [DIM, HW], xdt, isOutput=False)
    wqk_d = nc.declare_dram_parameter("wqk", [DIM, 128], xdt, isOutput=False)
    wv_d = nc.declare_dram_parameter("wv", [DIM, 64], xdt, isOutput=False)
    bqk_d = nc.declare_dram_parameter("bqk", [128, 1], f32, isOutput=False)
    bv_d = nc.declare_dram_parameter("bv", [64, 1], f32, isOutput=False)
    pdt = bf16 if POS_BF16 else f32r
    posuw_d = nc.declare_dram_parameter("posuw", [2, 2, DIM, SEQ], pdt, isOutput=False)
    ident_d = nc.declare_dram_parameter("ident", [128, 128], sdt, isOutput=False)
    out_d = nc.declare_dram_parameter("out", [DIM, HW], odt, isOutput=True)

    CHUNK = 8192  # pixels per pass-1 outer chunk
    SUB = 512  # pixels per projection matmul
    NSUB = CHUNK // SUB
    NCHUNK = HW // CHUNK

    def eng(i):  # alternate the two HWDGE rings
        return nc.sync if (i % 2 == 0) else nc.scalar

    with tile.TileContext(nc) as tc:
        with (
            tc.tile_pool(name="consts", bufs=1) as consts,
            tc.tile_pool(name="dram", bufs=1, space="DRAM") as dramp,
        ):
            spill = dramp.tile([DIM, 3, HW], sdt)  # [c, q/k/v, pix]

            wqk = consts.tile([DIM, 128], xdt)
            wv = consts.tile([DIM, 64], xdt)
            bqk = consts.tile([128, 1], f32)
            bv = consts.tile([64, 1], f32)
            ident = consts.tile([128, 128], sdt)
            nc.sync.dma_start(out=wqk[:], in_=wqk_d[:])
            nc.sync.dma_start(out=wv[:], in_=wv_d[:])
            nc.sync.dma_start(out=bqk[:], in_=bqk_d[:])
            nc.sync.dma_start(out=bv[:], in_=bv_d[:])
            nc.sync.dma_start(out=ident[:], in_=ident_d[:])

            for _rep in range(reps):
                # ------------ pass 1: QKV projection -> DRAM spill ----------
                if PASSES & 1:
                    with (
                      tc.tile_pool(name="p1x", bufs=3) as p1x,
                      tc.tile_pool(name="p1s", bufs=3) as p1s,
                      tc.tile_pool(name="p1ps", bufs=3, space="PSUM") as p1ps,
                  ):
                      for ch in range(NCHUNK):
                          pix0 = ch * CHUNK
                          xt = p1x.tile([DIM, CHUNK], xdt)
                          eng(ch).dma_start(out=xt[:], in_=x_d[:, pix0 : pix0 + CHUNK])
                          qk_st = p1s.tile([128, CHUNK], sdt)
                          v_st = p1s.tile([64, CHUNK], sdt)
                          for s in range(NSUB):
                              xs = xt[:, s * SUB : (s + 1) * SUB]
                              ps_qk = p1ps.tile([128, SUB], f32, tag="psqk")
                              ps_v = p1ps.tile([64, SUB], f32, tag="psv")
                              nc.tensor.matmul(ps_qk[:], wqk[:], xs)
                              nc.tensor.matmul(ps_v[:], wv[:], xs)
                              # psum -> sbuf (+bias, +cast): qk on DVE, v on ACT
                              nc.vector.tensor_scalar_add(
                                  qk_st[:, s * SUB : (s + 1) * SUB], ps_qk[:], bqk[:]
                              )
                              nc.scalar.activation(
                                  out=v_st[:, s * SUB : (s + 1) * SUB],
                                  in_=ps_v[:],
                                  func=AF.Identity,
                                  bias=bv[:],
                              )
                          eng(ch + 1).dma_start(
                              out=spill[:, 0, pix0 : pix0 + CHUNK],
                              in_=qk_st[0:64, :],
                          )
                          eng(ch).dma_start(
                              out=spill[:, 1, pix0 : pix0 + CHUNK],
                              in_=qk_st[64:128, :],
                          )
                          nc.gpsimd.dma_start(
                              out=spill[:, 2, pix0 : pix0 + CHUNK], in_=v_st[:]
                          )

                  # ------------ pass 2: per-slice attention -------------------
                if PASSES & 2:
                    with (
                      tc.tile_pool(name="p2io", bufs=3) as p2io,
                      tc.tile_pool(name="p2w", bufs=3) as p2w,
                      tc.tile_pool(name="p2pos", bufs=2) as p2pos,
                      tc.tile_pool(name="p2ps", bufs=2, space="PSUM") as p2ps,
                  ):
                      for c0 in range(0, DIM, SG):
                          gi = c0 // SG
                          pg = p2pos.tile([2, 2, SG, SEQ], pdt, tag="pg")
                          eng(gi + 1).dma_start(
                              out=pg[:], in_=posuw_d[:, :, c0 : c0 + SG, :]
                          )
                          # qkvg[p, ((cc*3 + s)*2 + g), w], h = g*128 + p
                          qkvg = p2io.tile([128, 6 * SG, SEQ], sdt, tag="qkvg")
                          eng(gi).dma_start(
                              out=qkvg[:],
                              in_=spill[:].rearrange(
                                  "c s (g p w) -> p (c s g) w", g=2, p=128
                              )[:, 6 * c0 : 6 * (c0 + SG), :],
                          )
                          og = p2io.tile([128, 2 * SG, SEQ], odt, tag="og")

                          for cc in range(SG):
                              c = c0 + cc
                              e = p2w.tile([128, 2, SEQ], sdt, tag="e")
                              rs = p2w.tile([128, 2], f32, tag="rs")
                              rr = p2w.tile([128, 2], f32, tag="rr")
                              for ic in range(2):
                                  ps_att = p2ps.tile([128, SEQ], f32, tag="psatt")
                                  isl = slice(ic * 128, (ic + 1) * 128)
                                  nc.tensor.matmul(
                                      ps_att[:], qkvg[:, cc * 6 + 0, isl],
                                      qkvg[:, cc * 6 + 2, :],
                                      start=True, stop=False,
                                  )
                                  nc.tensor.matmul(
                                      ps_att[:], qkvg[:, cc * 6 + 1, isl],
                                      qkvg[:, cc * 6 + 3, :],
                                      start=False, stop=False,
                                  )
                                  nc.tensor.matmul(
                                      ps_att[:], pg[:, 0, cc, isl], pg[:, 1, cc, :],
                                      start=False, stop=True,
                                  )
                                  nc.scalar.activation(
                                      out=e[:, ic, :],
                                      in_=ps_att[:],
                                      func=AF.Exp,
                                      scale=GAMMA,
                                      accum_out=rs[:, ic : ic + 1],
                                  )
                              nc.vector.reciprocal(rr[:], rs[:])
                              for ic in range(2):
                                  nc.vector.tensor_scalar_mul(
                                      e[:, ic, :], e[:, ic, :], rr[:, ic : ic + 1]
                                  )

                              # vT[p, wc, h] = v[c, h, wc*128+p] via PE transpose
                              vT = p2w.tile([128, 2, SEQ], sdt, tag="vT")
                              for g in range(2):
                                  for wc in range(2):
                                      ps_t = p2ps.tile([128, 128], sdt, tag="pst")
                                      nc.tensor.transpose(
                                          ps_t[:],
                                          qkvg[:, cc * 6 + 4 + g, wc * 128 : (wc + 1) * 128],
                                          ident[:],
                                      )
                                      nc.vector.tensor_copy(
                                          vT[:, wc, g * 128 : (g + 1) * 128], ps_t[:]
                                      )

                              # net[h,j] = sum_i v[h,i] e[i,j]
                              for m in range(2):
                                  ps_net = p2ps.tile([128, SEQ], f32, tag="psnet")
                                  msl = slice(m * 128, (m + 1) * 128)
                                  nc.tensor.matmul(
                                      ps_net[:], vT[:, 0, msl], e[:, 0, :],
                                      start=True, stop=False,
                                  )
                                  nc.tensor.matmul(
                                      ps_net[:], vT[:, 1, msl], e[:, 1, :],
                                      start=False, stop=True,
                                  )
                                  nc.vector.tensor_copy(og[:, cc * 2 + m, :], ps_net[:])
                          nc.gpsimd.dma_start(
                              out=out_d[:]
                              .rearrange("c (m p w) -> p (c m) w", m=2, p=128)[
                                  :, 2 * c0 : 2 * (c0 + SG), :
                              ],
                              in_=og[:],
                          )

    nc.compile()
    return nc


def _get_nc(reps=1):
    key = ("nc", reps)
    if key not in _CACHE:
        _CACHE[key] = _build_nc(reps)
    return _CACHE[key]


def _make_in_maps(x, W_qkv, b_qkv):
    import ml_dtypes

    posu, posw = _pos_factors()
    posuw = np.stack(
        [posu.transpose(1, 0, 2), posw.transpose(1, 0, 2)], axis=1
    )  # (2, 2, 64, 256)
    posuw = np.ascontiguousarray(posuw)
    if POS_BF16:
        posuw = posuw.astype(ml_dtypes.bfloat16)
    wdt = ml_dtypes.bfloat16 if X_BF16 else np.float32
    wqk = np.ascontiguousarray(W_qkv[0:128].T.astype(wdt))  # (64, 128)
    wv = np.ascontiguousarray(W_qkv[128:192].T.astype(wdt))  # (64, 64)

    def _xcast(a):
        return a.astype(ml_dtypes.bfloat16) if X_BF16 else a
    bqk = np.ascontiguousarray(b_qkv[0:128].astype(np.float32)).reshape(128, 1)
    bv = np.ascontiguousarray(b_qkv[128:192].astype(np.float32)).reshape(64, 1)
    ident = np.eye(128, dtype=np.float32)
    if SPILL_BF16:
        ident = ident.astype(ml_dtypes.bfloat16)
    in_maps = []
    for b in range(N_CORES):
        in_maps.append(
            {
                "x": _xcast(np.ascontiguousarray(x[b]).reshape(DIM, HW)),
                "wqk": wqk,
                "wv": wv,
                "bqk": bqk,
                "bv": bv,
                "posuw": posuw,
                "ident": ident,
            }
        )
    return in_maps


def kernel(x, W_qkv, b_qkv, pos):
    from concourse.bass_utils import run_bass_kernel_spmd

    x = np.asarray(x, dtype=np.float32)
    W_qkv = np.asarray(W_qkv, dtype=np.float32)
    b_qkv = np.asarray(b_qkv, dtype=np.float32)
    assert x.shape == (N_CORES, DIM, SEQ, SEQ)

    nc = _get_nc()
    in_maps = _make_in_maps(x, W_qkv, b_qkv)
    res = run_bass_kernel_spmd(nc, in_maps, core_ids=list(range(N_CORES)))
    out = np.stack(
        [
            np.asarray(res.results[b]["out"], dtype=np.float32).reshape(
                DIM, SEQ, SEQ
            )
            for b in range(N_CORES)
        ]
    )
    return out.astype(np.float32)


if __name__ == "__main__":
    nc = _get_nc()
    print("built ok")

